# revision 9
# baseline (speedup 1.0000x reference)
"""RSNN forward on 8 trn2 cores, data-parallel over batch.

Forward value of the reference == the no-grad spike recurrence `zs_rec`
(the dummy/surrogate path cancels in value). Pipeline:
  phase 1 (device): grouped causal input conv  -> a_conv [512, T] per batch el
  host: batch-norm stats across the 8 cores' a_conv (tiny), fold gamma/beta/
        bias into a per-channel threshold tensor rhs
  phase 2 (device): 512-step binary spike recurrence.

Phase 2 uses an exact bf16 multi-term weight split: W (fp32) = sum of
NTERMS bf16 matrices (residual ~2^-24 rel at 3 terms). z is binary 0/1,
so every bf16 product Wi*z is exact and the fp32 PSUM accumulation
reproduces the fp32 conv up to summation-order rounding. bf16
stationaries stream ~8x faster than fp32 into the PE array, which is the
bottleneck of the whole model. Taps 0-3 only read z older than one
block, so they are batched over 2-block supersteps (halving their weight
traffic); taps 4-7 run per block; the channel permutation runs as 16
bf16 0/1 matmuls (exact).
"""
import numpy as np
import ml_dtypes

import concourse.bass as bass
import concourse.mybir as mybir
from concourse.bass_utils import run_bass_kernel_spmd

F32 = mybir.dt.float32
BF16 = mybir.dt.bfloat16
H = 512; C_IN = 512; D = 4; K_REC = 8; K_IN = 8; G = 32
B, T = 8, 2048
S = T // D          # 512 recurrence blocks
U = S // 2          # 2-block supersteps
NCHUNK = 4          # 512 channels = 4 x 128 partitions
NTERMS = 2          # bf16 terms per fp32 weight
CORES = list(range(8))
BF = ml_dtypes.bfloat16


# ---------------- host-side weight packing ----------------

def _blockdiag_stationaries(w, K):
    """w: [512 out, 32 in, K] grouped (16 groups of 32->32).
    Returns [128, K*4, 128] stationary: col-block km=(k*4+m): S[p_in, km, p_out]
    = w[128m+p_out, p_in-32*(p_out//32), k] if same 32-block else 0."""
    out = np.zeros((128, K * 4, 128), np.float32)
    for k in range(K):
        for m in range(NCHUNK):
            blk = np.zeros((128, 128), np.float32)
            for g4 in range(4):                      # 4 groups per chunk
                h0 = 128 * m + 32 * g4
                blk[32*g4:32*g4+32, 32*g4:32*g4+32] = \
                    w[h0:h0+32, :, k].T              # [in 32, out 32]
            out[:, k * 4 + m, :] = blk
    return out


def _perm_stationaries(perm):
    """zp[j] = z[perm[j]] as matmul: SP[p_c, ic*4+oc, p_j] = 1 iff
    perm[128*oc+p_j] == 128*ic+p_c."""
    out = np.zeros((128, 16, 128), np.float32)
    for oc in range(NCHUNK):
        for j_p in range(128):
            c = int(perm[128 * oc + j_p])
            ic, c_p = c // 128, c % 128
            out[c_p, ic * 4 + oc, j_p] = 1.0
    return out


def _bf16_terms(w, n):
    """Split fp32 array into n bf16 terms summing to w (residual 2^-8n)."""
    terms = []
    r = w.astype(np.float32)
    for _ in range(n):
        t = r.astype(BF)
        terms.append(t)
        r = r - t.astype(np.float32)
    return terms


def _to_chunks(a):   # [512, X] -> [128, 4, X] with channel c = 128*q + p
    return np.ascontiguousarray(a.reshape(NCHUNK, 128, -1).transpose(1, 0, 2))


def _from_chunks(a):  # [128, 4, X] -> [512, X]
    return np.ascontiguousarray(a.transpose(1, 0, 2).reshape(512, -1))


# ---------------- phase 1: input conv ----------------

def _gen_phase1():
    nc = bass.Bass(target_bir_lowering=False)
    TP = T + K_IN - 1
    xin_d = nc.dram_tensor("xin", [128, NCHUNK * TP], F32, kind="ExternalInput")
    si_d = nc.dram_tensor("si", [128, K_IN * 4 * 128], F32, kind="ExternalInput")
    a_d = nc.dram_tensor("a_out", [128, NCHUNK * T], F32, kind="ExternalOutput")
    NT = T // 512

    with (
        nc.semaphore("dma_sem") as dma_sem,
        nc.semaphore("pe_sem") as pe_sem,
        nc.semaphore("dve_sem") as dve_sem,
        nc.semaphore("out_sem") as out_sem,
        nc.sbuf_tensor("xin_t", [128, NCHUNK, TP], F32) as xin_t,
        nc.sbuf_tensor("si_t", [128, K_IN * 4, 128], F32) as si_t,
        nc.sbuf_tensor("a_sb", [128, NCHUNK, T], F32) as a_sb,
        nc.psum_tensor("psA", [128, NT, 512], F32) as psA,
        nc.psum_tensor("psB", [128, NT, 512], F32) as psB,
    ):
        with nc.Block() as block:
            @block.sync
            def _(sync):
                sync.dma_start(xin_t[:, :, :], xin_d[:, :].rearrange(
                    "p (c t) -> p c t", c=NCHUNK)).then_inc(dma_sem, 16)
                sync.dma_start(si_t[:, :, :], si_d[:, :].rearrange(
                    "p (c t) -> p c t", c=K_IN * 4)).then_inc(dma_sem, 16)

            @block.tensor
            def _(tensor):
                tensor.wait_ge(dma_sem, 32)
                for m in range(NCHUNK):
                    PS = psA if m % 2 == 0 else psB
                    if m >= 2:
                        tensor.wait_ge(dve_sem, m - 1)
                    for k in range(K_IN):
                        for ct in range(NT):
                            mm = tensor.matmul(
                                PS[:, ct, :],
                                si_t[:, k * 4 + m, :],
                                xin_t[:, m, ct * 512 + k: ct * 512 + k + 512],
                                start=(k == 0), stop=(k == K_IN - 1),
                            )
                    mm.then_inc(pe_sem, 1)

            @block.vector
            def _(vector):
                for m in range(NCHUNK):
                    PS = psA if m % 2 == 0 else psB
                    vector.wait_ge(pe_sem, m + 1)
                    for ct in range(NT):
                        cp = vector.tensor_copy(
                            a_sb[:, m, ct * 512: ct * 512 + 512], PS[:, ct, :])
                    cp.then_inc(dve_sem, 1)

            @block.gpsimd
            def _(gpsimd):
                for m in range(NCHUNK):
                    gpsimd.wait_ge(dve_sem, m + 1)
                    gpsimd.dma_start(
                        a_d[:, m * T:(m + 1) * T], a_sb[:, m, :]
                    ).then_inc(out_sem, 16)
                gpsimd.wait_ge(out_sem, 64)
    return nc


# ---------------- phase 2: spike recurrence (bf16 multi-term) ----------------

def _gen_phase2(nterms=NTERMS, window8=True, ulim=U):
    nc = bass.Bass(target_bir_lowering=False)
    ZC = T + 24
    # zfe col c = z(c-12); zfo col c = z(c-13).  Two copies so every PE
    # moving-operand window starts at an even element (4B-aligned) offset:
    # tap k odd reads zfe[t+k+1 ...], tap k even reads zfo[t+k+2 ...].
    na_d = nc.dram_tensor("na_in", [128, NCHUNK * T], F32, kind="ExternalInput")
    w_d = {}
    for p in ("a", "b"):
        for i in range(nterms):
            w_d[p, i] = nc.dram_tensor(
                f"w{p}{i}", [128, K_REC * 4 * 128], BF16, kind="ExternalInput")
    sp_d = nc.dram_tensor("sp", [128, 16 * 128], BF16, kind="ExternalInput")
    z_d = nc.dram_tensor("z_out", [128, NCHUNK * T], BF16,
                         kind="ExternalOutput")

    with (
        nc.semaphore("dma_sem") as dma_sem,
        nc.semaphore("init_sem") as init_sem,
        nc.semaphore("pe_sem") as pe_sem,
        nc.semaphore("dve_sem") as dve_sem,
        nc.semaphore("out_sem") as out_sem,
        nc.sbuf_tensor("na_t", [128, NCHUNK, T], F32) as na_t,
        nc.sbuf_tensor("wts", [128, 2 * nterms * K_REC * 4, 128], BF16) as wts,
        nc.sbuf_tensor("sp_t", [128, 16, 128], BF16) as sp_t,
        nc.sbuf_tensor("zfe", [128, NCHUNK, ZC], BF16) as zfe,
        nc.sbuf_tensor("zfo", [128, NCHUNK, ZC], BF16) as zfo,
        nc.sbuf_tensor("zpe", [128, NCHUNK, ZC], BF16) as zpe,
        nc.sbuf_tensor("zpo", [128, NCHUNK, ZC], BF16) as zpo,
        nc.psum_tensor("pr0", [128, 512], F32) as pr0,
        nc.psum_tensor("pr1", [128, 512], F32) as pr1,
        nc.psum_tensor("pz0", [128, 512], F32) as pz0,
        nc.psum_tensor("pz1", [128, 512], F32) as pz1,
    ):
        # wts free-axis layout: [path(2) x term x K_REC x chunk]
        def wsl(p, th, k, m):
            i = ((p * nterms + th) * K_REC + k) * 4 + m
            return wts[:, i, :]

        def zwin(ze, zo, m, t, k, w):
            if k % 2 == 1:
                return ze[:, m, t + k + 1: t + k + 1 + w]
            return zo[:, m, t + k + 2: t + k + 2 + w]

        PR = [pr0, pr1]
        PZ = [pz0, pz1]
        NDMA = 16 * (2 + 2 * nterms)
        # dve_sem per block b: z-even ready at 4b+1, z-full (odd copy) at
        # 4b+2, zp-even at 4b+3, zp-full at 4b+4.
        with nc.Block() as block:
            @block.sync
            def _(sync):
                sync.dma_start(na_t[:, :, :], na_d[:, :].rearrange(
                    "p (c t) -> p c t", c=NCHUNK)).then_inc(dma_sem, 16)
                for p in range(2):
                    for th in range(nterms):
                        base = (p * nterms + th) * K_REC * 4
                        sync.dma_start(
                            wts[:, base: base + K_REC * 4, :],
                            w_d["ab"[p], th][:, :].rearrange(
                                "p (c t) -> p c t", c=K_REC * 4)
                        ).then_inc(dma_sem, 16)
                sync.dma_start(sp_t[:, :, :], sp_d[:, :].rearrange(
                    "p (c t) -> p c t", c=16)).then_inc(dma_sem, 16)

            @block.gpsimd
            def _(gpsimd):
                if ulim < U:   # bench mode: define full z_out region
                    gpsimd.memset(zfe[:, :, :], 0.0)
                    gpsimd.memset(zfo[:, :, :], 0.0)
                    gpsimd.memset(zpe[:, :, :], 0.0)
                    ms = gpsimd.memset(zpo[:, :, :], 0.0)
                else:
                    gpsimd.memset(zfe[:, :, 0:12], 0.0)
                    gpsimd.memset(zfo[:, :, 0:13], 0.0)
                    gpsimd.memset(zpe[:, :, 0:12], 0.0)
                    ms = gpsimd.memset(zpo[:, :, 0:13], 0.0)
                ms.then_inc(init_sem, 1)

            @block.tensor
            def _(tensor):
                tensor.wait_ge(dma_sem, NDMA)
                tensor.wait_ge(init_sem, 1)
                for u in range(ulim):
                    P = PR[u % 2]
                    t8 = 8 * u
                    # OLD-A: taps 0-3, A path, both blocks (z <= 2u-1)
                    if window8:
                        if u > 0:
                            tensor.wait_ge(dve_sem, 4 * (2 * u - 1) + 2)
                        for m in range(NCHUNK):
                            for k in range(4):
                                for th in range(nterms):
                                    tensor.matmul(
                                        P[:, 8 * m: 8 * m + 8], wsl(0, th, k, m),
                                        zwin(zfe, zfo, m, t8, k, 8),
                                        start=(m == 0 and k == 0 and th == 0),
                                        stop=False,
                                        skip_group_check=True)
                        # OLD-B needs zp-full(2u-1)
                        if u > 0:
                            tensor.wait_ge(dve_sem, 4 * (2 * u - 1) + 4)
                        for m in range(NCHUNK):
                            for k in range(4):
                                for th in range(nterms):
                                    tensor.matmul(
                                        P[:, 8 * m: 8 * m + 8], wsl(1, th, k, m),
                                        zwin(zpe, zpo, m, t8, k, 8),
                                        start=False, stop=False,
                                        skip_group_check=True)
                    for b2 in range(2):
                        b = 2 * u + b2
                        t4 = t8 + 4 * b2
                        # FRESH-A taps 4-7 (z-full of b-1)
                        if b > 0:
                            tensor.wait_ge(dve_sem, 4 * (b - 1) + 2)
                        ksA = range(4, 8) if window8 else range(8)
                        for m in range(NCHUNK):
                            c0 = 8 * m + 4 * b2
                            for k in ksA:
                                for th in range(nterms):
                                    tensor.matmul(
                                        P[:, c0: c0 + 4], wsl(0, th, k, m),
                                        zwin(zfe, zfo, m, t4, k, 4),
                                        start=(not window8 and b2 == 0
                                               and m == 0 and k == ksA[0]
                                               and th == 0),
                                        stop=False,
                                        skip_group_check=True)
                        # FRESH-B (zp-full of b-1)
                        if b > 0:
                            tensor.wait_ge(dve_sem, 4 * (b - 1) + 4)
                        ksB = range(4, 8) if window8 else range(8)
                        for m in range(NCHUNK):
                            c0 = 8 * m + 4 * b2
                            for k in ksB:
                                for th in range(nterms):
                                    mm = tensor.matmul(
                                        P[:, c0: c0 + 4], wsl(1, th, k, m),
                                        zwin(zpe, zpo, m, t4, k, 4),
                                        start=False,
                                        stop=(k == 7 and th == nterms - 1),
                                        skip_group_check=True)
                        mm.then_inc(pe_sem, 1)
                        # PERM(b): needs z-even(b)
                        tensor.wait_ge(dve_sem, 4 * b + 1)
                        for oc in range(NCHUNK):
                            for ic in range(NCHUNK):
                                mm = tensor.matmul(
                                    PZ[b % 2][:, 4 * oc: 4 * oc + 4],
                                    sp_t[:, ic * 4 + oc, :],
                                    zfe[:, ic, t4 + 12: t4 + 16],
                                    start=(oc == 0 and ic == 0),
                                    stop=(oc == NCHUNK - 1
                                          and ic == NCHUNK - 1),
                                    skip_group_check=True)
                        mm.then_inc(pe_sem, 1)

            @block.vector
            def _(vector):
                vector.wait_ge(dma_sem, NDMA)
                for b in range(2 * ulim):
                    u, b2 = b // 2, b % 2
                    t4 = 4 * b
                    vector.wait_ge(pe_sem, 2 * b + 1)
                    for m in range(NCHUNK):
                        op = vector.tensor_tensor(
                            zfe[:, m, t4 + 12: t4 + 16],
                            PR[u % 2][:, 8 * m + 4 * b2: 8 * m + 4 * b2 + 4],
                            na_t[:, m, t4: t4 + 4],
                            mybir.AluOpType.is_gt)
                    vector.drain().then_inc(dve_sem, 1)
                    for m in range(NCHUNK):
                        op = vector.tensor_tensor(
                            zfo[:, m, t4 + 13: t4 + 17],
                            PR[u % 2][:, 8 * m + 4 * b2: 8 * m + 4 * b2 + 4],
                            na_t[:, m, t4: t4 + 4],
                            mybir.AluOpType.is_gt)
                    vector.drain().then_inc(dve_sem, 1)
                    vector.wait_ge(pe_sem, 2 * b + 2)
                    for oc in range(NCHUNK):
                        op = vector.tensor_copy(
                            zpe[:, oc, t4 + 12: t4 + 16],
                            PZ[b % 2][:, 4 * oc: 4 * oc + 4])
                    vector.drain().then_inc(dve_sem, 1)
                    for oc in range(NCHUNK):
                        op = vector.tensor_copy(
                            zpo[:, oc, t4 + 13: t4 + 17],
                            PZ[b % 2][:, 4 * oc: 4 * oc + 4])
                    vector.drain().then_inc(dve_sem, 1)

            @block.gpsimd
            def _(gpsimd):
                gpsimd.wait_ge(dve_sem, 8 * ulim)
                gpsimd.dma_start(
                    z_d[:, :].rearrange("p (c t) -> p c t", c=NCHUNK),
                    zfe[:, :, 12: 12 + T],
                ).then_inc(out_sem, 16)
                gpsimd.wait_ge(out_sem, 16)
    return nc


# ---------------- top level ----------------

_timings = {}


def kernel(x, w_in, w_rec, bn_gamma, bn_beta, bias, perm_in, perm_rec):
    import time as _time
    x = np.asarray(x, np.float32)
    w_in = np.asarray(w_in, np.float32)
    w_rec = np.asarray(w_rec, np.float32)
    bn_gamma = np.asarray(bn_gamma, np.float32)
    bn_beta = np.asarray(bn_beta, np.float32)
    bias = np.asarray(bias, np.float32)
    perm_in = np.asarray(perm_in).astype(np.int64)
    perm_rec = np.asarray(perm_rec).astype(np.int64)

    # phase 1 inputs
    si = _blockdiag_stationaries(w_in, K_IN).reshape(128, -1)
    in_maps1 = []
    for b in range(B):
        xt = x[b].T                               # [512, T]
        xp = np.concatenate(
            [np.zeros((C_IN, K_IN - 1), np.float32), xt[perm_in]], axis=1)
        in_maps1.append({"xin": _to_chunks(xp).reshape(128, -1), "si": si})

    _t = _time.time()
    r1 = run_bass_kernel_spmd(_gen_phase1(), in_maps1, CORES)
    _timings['phase1_wall_s'] = _time.time() - _t
    a_conv = np.stack([
        _from_chunks(r1.results[c]["a_out"].reshape(128, NCHUNK, T))
        for c in range(B)])                       # [B, 512, T]

    # batch-norm stats (training mode, biased) + fold bias into threshold:
    # z = (a_rec > -(gamma*(a-mu)*r + beta + bias)) = (a_rec > rhs)
    # with rhs = (-s_c)*a_conv + (s_c*mu - beta - bias)
    mu = np.mean(a_conv, axis=(0, 2), dtype=np.float32)
    var = np.mean((a_conv - mu[None, :, None]) ** 2, axis=(0, 2),
                  dtype=np.float32)
    r = (1.0 / np.sqrt(var + np.float32(1e-5))).astype(np.float32)
    s_c = (bn_gamma * r).astype(np.float32)
    shift = (s_c * mu - bn_beta - bias).astype(np.float32)

    # phase 2 weights: exact bf16 term split of the fp32 stationaries
    wA = w_rec[:, 0::2, :]                        # identity path
    wB = w_rec[:, 1::2, :]                        # permuted path
    wa_f = _blockdiag_stationaries(wA, K_REC)
    wb_f = _blockdiag_stationaries(wB, K_REC)
    wa_terms = _bf16_terms(wa_f, NTERMS)
    wb_terms = _bf16_terms(wb_f, NTERMS)
    sp = _perm_stationaries(perm_rec).reshape(128, -1).astype(BF)

    in_maps2 = []
    for b in range(B):
        rhs = (-s_c)[:, None] * a_conv[b] + shift[:, None]  # [512, T]
        im = {"na_in": _to_chunks(rhs.astype(np.float32)).reshape(128, -1),
              "sp": sp}
        for i in range(NTERMS):
            im[f"wa{i}"] = np.ascontiguousarray(wa_terms[i]).reshape(128, -1)
            im[f"wb{i}"] = np.ascontiguousarray(wb_terms[i]).reshape(128, -1)
        in_maps2.append(im)
    _t = _time.time()
    r2 = run_bass_kernel_spmd(_gen_phase2(NTERMS), in_maps2, CORES)
    _timings['phase2_wall_s'] = _time.time() - _t

    out = np.stack([
        _from_chunks(r2.results[c]["z_out"].reshape(
            128, NCHUNK, T).astype(np.float32)).T
        for c in range(B)])                       # [B, T, 512]
    return np.ascontiguousarray(out.astype(np.float32))


# revision 10
# speedup vs baseline: 1.4788x; 1.4788x over previous
"""RSNN forward on 8 trn2 cores, data-parallel over batch.

Forward value of the reference == the no-grad spike recurrence `zs_rec`
(the dummy/surrogate path cancels in value). Pipeline:
  phase 1 (device): grouped causal input conv  -> a_conv [512, T] per batch el
  host: batch-norm stats across the 8 cores' a_conv (tiny), fold gamma/beta/
        bias into a per-channel threshold tensor rhs
  phase 2 (device): 512-step binary spike recurrence.

Phase 2 uses an exact bf16 multi-term weight split: W (fp32) = sum of
NTERMS bf16 matrices (residual ~2^-24 rel at 3 terms). z is binary 0/1,
so every bf16 product Wi*z is exact and the fp32 PSUM accumulation
reproduces the fp32 conv up to summation-order rounding. bf16
stationaries stream ~8x faster than fp32 into the PE array, which is the
bottleneck of the whole model. Taps 0-3 only read z older than one
block, so they are batched over 2-block supersteps (halving their weight
traffic); taps 4-7 run per block; the channel permutation runs as 16
bf16 0/1 matmuls (exact).
"""
import numpy as np
import ml_dtypes

import concourse.bass as bass
import concourse.mybir as mybir
from concourse.bass_utils import run_bass_kernel_spmd

F32 = mybir.dt.float32
BF16 = mybir.dt.bfloat16
H = 512; C_IN = 512; D = 4; K_REC = 8; K_IN = 8; G = 32
B, T = 8, 2048
S = T // D          # 512 recurrence blocks
U = S // 2          # 2-block supersteps
NCHUNK = 4          # 512 channels = 4 x 128 partitions
NTERMS = 2          # bf16 terms per fp32 weight
CORES = list(range(8))
BF = ml_dtypes.bfloat16


# ---------------- host-side weight packing ----------------

def _blockdiag_stationaries(w, K):
    """w: [512 out, 32 in, K] grouped (16 groups of 32->32).
    Returns [128, K*4, 128] stationary: col-block km=(k*4+m): S[p_in, km, p_out]
    = w[128m+p_out, p_in-32*(p_out//32), k] if same 32-block else 0."""
    out = np.zeros((128, K * 4, 128), np.float32)
    for k in range(K):
        for m in range(NCHUNK):
            blk = np.zeros((128, 128), np.float32)
            for g4 in range(4):                      # 4 groups per chunk
                h0 = 128 * m + 32 * g4
                blk[32*g4:32*g4+32, 32*g4:32*g4+32] = \
                    w[h0:h0+32, :, k].T              # [in 32, out 32]
            out[:, k * 4 + m, :] = blk
    return out


def _perm_stationaries(perm):
    """zp[j] = z[perm[j]] as matmul: SP[p_c, ic*4+oc, p_j] = 1 iff
    perm[128*oc+p_j] == 128*ic+p_c."""
    out = np.zeros((128, 16, 128), np.float32)
    for oc in range(NCHUNK):
        for j_p in range(128):
            c = int(perm[128 * oc + j_p])
            ic, c_p = c // 128, c % 128
            out[c_p, ic * 4 + oc, j_p] = 1.0
    return out


def _bf16_terms(w, n):
    """Split fp32 array into n bf16 terms summing to w (residual 2^-8n)."""
    terms = []
    r = w.astype(np.float32)
    for _ in range(n):
        t = r.astype(BF)
        terms.append(t)
        r = r - t.astype(np.float32)
    return terms


def _to_chunks(a):   # [512, X] -> [128, 4, X] with channel c = 128*q + p
    return np.ascontiguousarray(a.reshape(NCHUNK, 128, -1).transpose(1, 0, 2))


def _from_chunks(a):  # [128, 4, X] -> [512, X]
    return np.ascontiguousarray(a.transpose(1, 0, 2).reshape(512, -1))


# ---------------- phase 1: input conv ----------------

def _gen_phase1():
    nc = bass.Bass(target_bir_lowering=False)
    TP = T + K_IN - 1
    xin_d = nc.dram_tensor("xin", [128, NCHUNK * TP], F32, kind="ExternalInput")
    si_d = nc.dram_tensor("si", [128, K_IN * 4 * 128], F32, kind="ExternalInput")
    a_d = nc.dram_tensor("a_out", [128, NCHUNK * T], F32, kind="ExternalOutput")
    NT = T // 512

    with (
        nc.semaphore("dma_sem") as dma_sem,
        nc.semaphore("pe_sem") as pe_sem,
        nc.semaphore("dve_sem") as dve_sem,
        nc.semaphore("out_sem") as out_sem,
        nc.sbuf_tensor("xin_t", [128, NCHUNK, TP], F32) as xin_t,
        nc.sbuf_tensor("si_t", [128, K_IN * 4, 128], F32) as si_t,
        nc.sbuf_tensor("a_sb", [128, NCHUNK, T], F32) as a_sb,
        nc.psum_tensor("psA", [128, NT, 512], F32) as psA,
        nc.psum_tensor("psB", [128, NT, 512], F32) as psB,
    ):
        with nc.Block() as block:
            @block.sync
            def _(sync):
                sync.dma_start(xin_t[:, :, :], xin_d[:, :].rearrange(
                    "p (c t) -> p c t", c=NCHUNK)).then_inc(dma_sem, 16)
                sync.dma_start(si_t[:, :, :], si_d[:, :].rearrange(
                    "p (c t) -> p c t", c=K_IN * 4)).then_inc(dma_sem, 16)

            @block.tensor
            def _(tensor):
                tensor.wait_ge(dma_sem, 32)
                for m in range(NCHUNK):
                    PS = psA if m % 2 == 0 else psB
                    if m >= 2:
                        tensor.wait_ge(dve_sem, m - 1)
                    for k in range(K_IN):
                        for ct in range(NT):
                            mm = tensor.matmul(
                                PS[:, ct, :],
                                si_t[:, k * 4 + m, :],
                                xin_t[:, m, ct * 512 + k: ct * 512 + k + 512],
                                start=(k == 0), stop=(k == K_IN - 1),
                            )
                    mm.then_inc(pe_sem, 1)

            @block.vector
            def _(vector):
                for m in range(NCHUNK):
                    PS = psA if m % 2 == 0 else psB
                    vector.wait_ge(pe_sem, m + 1)
                    for ct in range(NT):
                        cp = vector.tensor_copy(
                            a_sb[:, m, ct * 512: ct * 512 + 512], PS[:, ct, :])
                    cp.then_inc(dve_sem, 1)

            @block.gpsimd
            def _(gpsimd):
                for m in range(NCHUNK):
                    gpsimd.wait_ge(dve_sem, m + 1)
                    gpsimd.dma_start(
                        a_d[:, m * T:(m + 1) * T], a_sb[:, m, :]
                    ).then_inc(out_sem, 16)
                gpsimd.wait_ge(out_sem, 64)
    return nc


# ---------------- phase 2: spike recurrence (bf16 multi-term) ----------------

def _gen_phase2(nterms=NTERMS, window8=True, ulim=U):
    nc = bass.Bass(target_bir_lowering=False)
    ZC = T + 24
    # zfe col c = z(c-12); zfo col c = z(c-13).  Two copies so every PE
    # moving-operand window starts at an even element (4B-aligned) offset:
    # tap k odd reads zfe[t+k+1 ...], tap k even reads zfo[t+k+2 ...].
    na_d = nc.dram_tensor("na_in", [128, NCHUNK * T], F32, kind="ExternalInput")
    w_d = {}
    for p in ("a", "b"):
        for i in range(nterms):
            w_d[p, i] = nc.dram_tensor(
                f"w{p}{i}", [128, K_REC * 4 * 128], BF16, kind="ExternalInput")
    sp_d = nc.dram_tensor("sp", [128, 16 * 128], BF16, kind="ExternalInput")
    z_d = nc.dram_tensor("z_out", [128, NCHUNK * T], BF16,
                         kind="ExternalOutput")

    with (
        nc.semaphore("dma_sem") as dma_sem,
        nc.semaphore("init_sem") as init_sem,
        nc.semaphore("pe_sem") as pe_sem,
        nc.semaphore("dve_sem") as dve_sem,
        nc.semaphore("out_sem") as out_sem,
        nc.sbuf_tensor("na_t", [128, NCHUNK, T], F32) as na_t,
        nc.sbuf_tensor("wts", [128, 2 * nterms * K_REC * 4, 128], BF16) as wts,
        nc.sbuf_tensor("sp_t", [128, 16, 128], BF16) as sp_t,
        nc.sbuf_tensor("zfe", [128, NCHUNK, ZC], BF16) as zfe,
        nc.sbuf_tensor("zfo", [128, NCHUNK, ZC], BF16) as zfo,
        nc.sbuf_tensor("zpe", [128, NCHUNK, ZC], BF16) as zpe,
        nc.sbuf_tensor("zpo", [128, NCHUNK, ZC], BF16) as zpo,
        nc.psum_tensor("pr0", [128, 512], F32) as pr0,
        nc.psum_tensor("pr1", [128, 512], F32) as pr1,
        nc.psum_tensor("pz0", [128, 512], F32) as pz0,
        nc.psum_tensor("pz1", [128, 512], F32) as pz1,
    ):
        # wts free-axis layout: [path(2) x term x K_REC x chunk]
        def wsl(p, th, k, m):
            i = ((p * nterms + th) * K_REC + k) * 4 + m
            return wts[:, i, :]

        def zwin(ze, zo, m, t, k, w):
            if k % 2 == 1:
                return ze[:, m, t + k + 1: t + k + 1 + w]
            return zo[:, m, t + k + 2: t + k + 2 + w]

        PR = [pr0, pr1]
        PZ = [pz0, pz1]
        NDMA = 16 * (2 + 2 * nterms)
        # dve_sem per block b: z-even ready at 4b+1, z-full (odd copy) at
        # 4b+2, zp-even at 4b+3, zp-full at 4b+4.
        with nc.Block() as block:
            @block.sync
            def _(sync):
                sync.dma_start(na_t[:, :, :], na_d[:, :].rearrange(
                    "p (c t) -> p c t", c=NCHUNK)).then_inc(dma_sem, 16)
                for p in range(2):
                    for th in range(nterms):
                        base = (p * nterms + th) * K_REC * 4
                        sync.dma_start(
                            wts[:, base: base + K_REC * 4, :],
                            w_d["ab"[p], th][:, :].rearrange(
                                "p (c t) -> p c t", c=K_REC * 4)
                        ).then_inc(dma_sem, 16)
                sync.dma_start(sp_t[:, :, :], sp_d[:, :].rearrange(
                    "p (c t) -> p c t", c=16)).then_inc(dma_sem, 16)

            @block.gpsimd
            def _(gpsimd):
                if ulim < U:   # bench mode: define full z_out region
                    gpsimd.memset(zfe[:, :, :], 0.0)
                    gpsimd.memset(zfo[:, :, :], 0.0)
                    gpsimd.memset(zpe[:, :, :], 0.0)
                    ms = gpsimd.memset(zpo[:, :, :], 0.0)
                else:
                    gpsimd.memset(zfe[:, :, 0:12], 0.0)
                    gpsimd.memset(zfo[:, :, 0:13], 0.0)
                    gpsimd.memset(zpe[:, :, 0:12], 0.0)
                    ms = gpsimd.memset(zpo[:, :, 0:13], 0.0)
                ms.then_inc(init_sem, 1)

            @block.tensor
            def _(tensor):
                tensor.wait_ge(dma_sem, NDMA)
                tensor.wait_ge(init_sem, 1)
                for u in range(ulim):
                    P = PR[u % 2]
                    t8 = 8 * u
                    # OLD-A: taps 0-3, A path, both blocks (z <= 2u-1)
                    if window8:
                        if u > 0:
                            tensor.wait_ge(dve_sem, 4 * (2 * u - 1) + 2)
                        for m in range(NCHUNK):
                            for k in range(4):
                                for th in range(nterms):
                                    tensor.matmul(
                                        P[:, 8 * m: 8 * m + 8], wsl(0, th, k, m),
                                        zwin(zfe, zfo, m, t8, k, 8),
                                        start=(m == 0 and k == 0 and th == 0),
                                        stop=False,
                                        skip_group_check=True)
                        # OLD-B needs zp-full(2u-1)
                        if u > 0:
                            tensor.wait_ge(dve_sem, 4 * (2 * u - 1) + 4)
                        for m in range(NCHUNK):
                            for k in range(4):
                                for th in range(nterms):
                                    tensor.matmul(
                                        P[:, 8 * m: 8 * m + 8], wsl(1, th, k, m),
                                        zwin(zpe, zpo, m, t8, k, 8),
                                        start=False, stop=False,
                                        skip_group_check=True)
                    for b2 in range(2):
                        b = 2 * u + b2
                        t4 = t8 + 4 * b2
                        # FRESH-A taps 4-7 (z-full of b-1)
                        if b > 0:
                            tensor.wait_ge(dve_sem, 4 * (b - 1) + 2)
                        ksA = range(4, 8) if window8 else range(8)
                        for m in range(NCHUNK):
                            c0 = 8 * m + 4 * b2
                            for k in ksA:
                                for th in range(nterms):
                                    tensor.matmul(
                                        P[:, c0: c0 + 4], wsl(0, th, k, m),
                                        zwin(zfe, zfo, m, t4, k, 4),
                                        start=(not window8 and b2 == 0
                                               and m == 0 and k == ksA[0]
                                               and th == 0),
                                        stop=False,
                                        skip_group_check=True)
                        # FRESH-B (zp-full of b-1)
                        if b > 0:
                            tensor.wait_ge(dve_sem, 4 * (b - 1) + 4)
                        ksB = range(4, 8) if window8 else range(8)
                        for m in range(NCHUNK):
                            c0 = 8 * m + 4 * b2
                            for k in ksB:
                                for th in range(nterms):
                                    mm = tensor.matmul(
                                        P[:, c0: c0 + 4], wsl(1, th, k, m),
                                        zwin(zpe, zpo, m, t4, k, 4),
                                        start=False,
                                        stop=(k == 7 and th == nterms - 1),
                                        skip_group_check=True)
                        mm.then_inc(pe_sem, 1)
                        # PERM(b): needs z-even(b)
                        tensor.wait_ge(dve_sem, 4 * b + 1)
                        for oc in range(NCHUNK):
                            for ic in range(NCHUNK):
                                mm = tensor.matmul(
                                    PZ[b % 2][:, 4 * oc: 4 * oc + 4],
                                    sp_t[:, ic * 4 + oc, :],
                                    zfe[:, ic, t4 + 12: t4 + 16],
                                    start=(oc == 0 and ic == 0),
                                    stop=(oc == NCHUNK - 1
                                          and ic == NCHUNK - 1),
                                    skip_group_check=True)
                        mm.then_inc(pe_sem, 1)

            @block.vector
            def _(vector):
                vector.wait_ge(dma_sem, NDMA)
                for b in range(2 * ulim):
                    u, b2 = b // 2, b % 2
                    t4 = 4 * b
                    vector.wait_ge(pe_sem, 2 * b + 1)
                    # fused across chunks: psum view [128, m(stride 8), 4]
                    prv = PR[u % 2][:, 0:32].rearrange(
                        "p (m c) -> p m c", m=NCHUNK)[:, :, 4 * b2: 4 * b2 + 4]
                    vector.tensor_tensor(
                        zfe[:, :, t4 + 12: t4 + 16], prv,
                        na_t[:, :, t4: t4 + 4], mybir.AluOpType.is_gt)
                    vector.drain().then_inc(dve_sem, 1)
                    vector.tensor_tensor(
                        zfo[:, :, t4 + 13: t4 + 17], prv,
                        na_t[:, :, t4: t4 + 4], mybir.AluOpType.is_gt)
                    vector.drain().then_inc(dve_sem, 1)
                    vector.wait_ge(pe_sem, 2 * b + 2)
                    pzv = PZ[b % 2][:, 0:16].rearrange(
                        "p (oc c) -> p oc c", oc=NCHUNK)
                    vector.tensor_copy(zpe[:, :, t4 + 12: t4 + 16], pzv)
                    vector.drain().then_inc(dve_sem, 1)
                    vector.tensor_copy(zpo[:, :, t4 + 13: t4 + 17], pzv)
                    vector.drain().then_inc(dve_sem, 1)

            @block.gpsimd
            def _(gpsimd):
                gpsimd.wait_ge(dve_sem, 8 * ulim)
                gpsimd.dma_start(
                    z_d[:, :].rearrange("p (c t) -> p c t", c=NCHUNK),
                    zfe[:, :, 12: 12 + T],
                ).then_inc(out_sem, 16)
                gpsimd.wait_ge(out_sem, 16)
    return nc


# ---------------- top level ----------------

_timings = {}


def kernel(x, w_in, w_rec, bn_gamma, bn_beta, bias, perm_in, perm_rec):
    import time as _time
    x = np.asarray(x, np.float32)
    w_in = np.asarray(w_in, np.float32)
    w_rec = np.asarray(w_rec, np.float32)
    bn_gamma = np.asarray(bn_gamma, np.float32)
    bn_beta = np.asarray(bn_beta, np.float32)
    bias = np.asarray(bias, np.float32)
    perm_in = np.asarray(perm_in).astype(np.int64)
    perm_rec = np.asarray(perm_rec).astype(np.int64)

    # phase 1 inputs
    si = _blockdiag_stationaries(w_in, K_IN).reshape(128, -1)
    in_maps1 = []
    for b in range(B):
        xt = x[b].T                               # [512, T]
        xp = np.concatenate(
            [np.zeros((C_IN, K_IN - 1), np.float32), xt[perm_in]], axis=1)
        in_maps1.append({"xin": _to_chunks(xp).reshape(128, -1), "si": si})

    _t = _time.time()
    r1 = run_bass_kernel_spmd(_gen_phase1(), in_maps1, CORES)
    _timings['phase1_wall_s'] = _time.time() - _t
    a_conv = np.stack([
        _from_chunks(r1.results[c]["a_out"].reshape(128, NCHUNK, T))
        for c in range(B)])                       # [B, 512, T]

    # batch-norm stats (training mode, biased) + fold bias into threshold:
    # z = (a_rec > -(gamma*(a-mu)*r + beta + bias)) = (a_rec > rhs)
    # with rhs = (-s_c)*a_conv + (s_c*mu - beta - bias)
    mu = np.mean(a_conv, axis=(0, 2), dtype=np.float32)
    var = np.mean((a_conv - mu[None, :, None]) ** 2, axis=(0, 2),
                  dtype=np.float32)
    r = (1.0 / np.sqrt(var + np.float32(1e-5))).astype(np.float32)
    s_c = (bn_gamma * r).astype(np.float32)
    shift = (s_c * mu - bn_beta - bias).astype(np.float32)

    # phase 2 weights: exact bf16 term split of the fp32 stationaries
    wA = w_rec[:, 0::2, :]                        # identity path
    wB = w_rec[:, 1::2, :]                        # permuted path
    wa_f = _blockdiag_stationaries(wA, K_REC)
    wb_f = _blockdiag_stationaries(wB, K_REC)
    wa_terms = _bf16_terms(wa_f, NTERMS)
    wb_terms = _bf16_terms(wb_f, NTERMS)
    sp = _perm_stationaries(perm_rec).reshape(128, -1).astype(BF)

    in_maps2 = []
    for b in range(B):
        rhs = (-s_c)[:, None] * a_conv[b] + shift[:, None]  # [512, T]
        im = {"na_in": _to_chunks(rhs.astype(np.float32)).reshape(128, -1),
              "sp": sp}
        for i in range(NTERMS):
            im[f"wa{i}"] = np.ascontiguousarray(wa_terms[i]).reshape(128, -1)
            im[f"wb{i}"] = np.ascontiguousarray(wb_terms[i]).reshape(128, -1)
        in_maps2.append(im)
    _t = _time.time()
    r2 = run_bass_kernel_spmd(_gen_phase2(NTERMS), in_maps2, CORES)
    _timings['phase2_wall_s'] = _time.time() - _t

    out = np.stack([
        _from_chunks(r2.results[c]["z_out"].reshape(
            128, NCHUNK, T).astype(np.float32)).T
        for c in range(B)])                       # [B, T, 512]
    return np.ascontiguousarray(out.astype(np.float32))
